# revision 4
# baseline (speedup 1.0000x reference)
"""AsymmetricSVD segment-reduce kernel for 8 TRN2 NeuronCores.

Strategy (data-parallel over segments, fp8 + DoubleRow):
  - Core m owns segments [512m, 512(m+1)) and their contiguous implicit
    entries (segment_ids is sorted).
  - Host precomputes per-entry scalar a_e = r_e - MU - bu[user[seg_e]] and a
    fused fp8 table XY = 128*[X | Y - bi*X] (so w*X + Y == a*X + Y'; the
    2^7 scale keeps fp8e4 out of subnormals and is folded back in Qn2).
  - Entries are bucketed by item range (4 buckets of 25000 rows so gather
    indices fit int16) and, within a bucket, grouped by 64-segment
    superblock.  Each (bucket, superblock) run is padded to a multiple of
    256 entries (cross-core max capacity, so the compiled graph is uniform
    across cores) so every 256-entry PAIR of gather groups lies inside one
    superblock -> one PSUM region (bank sb, rows 0:64 -- DoubleRow requires
    PSUM base partition 0).
  - Device gathers 256B fp8 rows per entry via gpsimd.dma_gather (SWDGE),
    4 queues, ~1550-descriptor calls (2 per run; must stay under the
    ~2048-desc ring capacity or the single Q7 blocks inside one call
    while the other queues run dry).  The SWDGE random-gather wall is ~2ns per
    256B descriptor aggregate; descriptor count, not bytes, dominates.
  - The one-hot/coefficient lhsT tiles are PRE-BUILT ON HOST in fp8,
    group-major ([128, groups, 2, 64]: Sp, S per 128-entry group; in-run
    duplicate items merged into multi-hot columns) and streamed via HWDGE -
    no on-device one-hot construction at all.
  - Adjacent group pairs run as fp8 DoubleRow matmuls (256 entries each,
    0.5 cyc/row, k-tiles = the two groups); odd trailing groups of a run
    use plain fp8 matmuls:
        PSUM[sb][0:64, 0:128]   += sum_e a_e * X_e      (lhsT = Sp)
        PSUM[sb][0:64, 128:256] += sum_e (Y')_e         (lhsT = S)
  - Chunks run superblock-major across buckets, so each superblock's
    accumulation closes and its epilogue runs while later superblocks are
    still gathering.
  - Epilogue: rui[seg] = bui[seg] + reduce_add(PSUM[seg, 0:256] * Qn2[seg])
    with Qn2 = [Qn | Qn], Qn = Q[item]*norm/128 precomputed on host.

Measured: ~262us HW exec on 8 cores, rel err ~1.7e-5.
"""

import numpy as np
import ml_dtypes

MU = 3.5
B = 4096
F = 128
NUM_ITEMS = 100000
N_CORES = 8
SEGS_PER_CORE = B // N_CORES            # 512
N_BUCKETS = 4
BUCKET_ROWS = (NUM_ITEMS + N_BUCKETS - 1) // N_BUCKETS   # 25000 < 32768 (int16)
SB = 128                                 # segments per superblock
NSB = SEGS_PER_CORE // SB                # 4 superblocks per core
PAIR = 256                               # entries per DoubleRow matmul pair
# Gather calls must stay under the ~2048-descriptor SWDGE ring capacity:
# larger calls block the single Q7 inside one call while the other queues
# run dry (measured 20us stalls per oversized call).
CHUNK = 1664                             # -> 2 calls per run (~1550 descs),
                                         # under the ~2048-desc ring capacity
N_QUEUES = 4                             # SWDGE gather queues (ucode max)
FP8 = ml_dtypes.float8_e4m3
XSCALE = 128.0                           # fp8 range scale for X/Y' rows


def _host_prep(bu, bi, Q, X, Y, user, item, imp_items, imp_ratings, segment_ids):
    """All index/scalar preprocessing. Returns per-core device arrays and
    uniform cross-core metadata for codegen."""
    a_full = imp_ratings.astype(np.float32) - MU - bu[user[segment_ids], 0]
    Yp = Y - bi * X                                    # [NUM_ITEMS, F]
    XYs = np.concatenate([X * XSCALE, Yp * XSCALE], axis=1).astype(FP8)

    counts = np.bincount(segment_ids, minlength=B).astype(np.float32)
    norm = np.where(counts > 0, counts, 1.0) ** -0.5
    bui = (MU + bu[user, 0] + bi[item, 0]).astype(np.float32)          # [B]
    Qh = (Q[item] * (norm / XSCALE)[:, None]).astype(np.float32)       # [B, F]
    Qn2 = np.concatenate([Qh, Qh], axis=1)                             # [B, 256]

    # --- shard entries by segment block; group by (bucket, superblock).
    # Keep segment order within runs: ascending-item (HBM-sorted) descriptor
    # order measured SLOWER (channel serialization), so don't sort by item.
    bounds = np.searchsorted(segment_ids, np.arange(0, B + 1, SEGS_PER_CORE))
    percore = []
    cnt = np.zeros((N_CORES, N_BUCKETS, NSB), np.int64)
    for m in range(N_CORES):
        lo, hi = bounds[m], bounds[m + 1]
        it = imp_items[lo:hi]
        sl = (segment_ids[lo:hi] - m * SEGS_PER_CORE).astype(np.int64)
        av = a_full[lo:hi]
        bk = it // BUCKET_ROWS
        key = bk * NSB + sl // SB
        order = np.argsort(key, kind="stable")
        it, sl, av, key = it[order], sl[order], av[order], key[order]
        ne = it.shape[0]
        # merge duplicate items within a run (multi-hot lhsT column, one
        # gather descriptor), keeping first-occurrence order so descriptor
        # addresses stay random (item-sorted order measured slower).
        pos = np.arange(ne)
        o2 = np.lexsort((pos, it, key))
        k2, i2, p2 = key[o2], it[o2], pos[o2]
        new = np.ones(ne, bool)
        new[1:] = (k2[1:] != k2[:-1]) | (i2[1:] != i2[:-1])
        gid = np.cumsum(new) - 1
        first_of = np.empty(ne, np.int64)
        first_of[o2] = p2[new][gid]          # first occurrence position
        rep = pos == first_of                # representative entries
        cnt[m] = np.bincount(key[rep], minlength=N_BUCKETS * NSB).reshape(
            N_BUCKETS, NSB)
        percore.append((it, sl, av, key, rep, first_of))

    # uniform per-(bucket, superblock) capacity: cross-core max, rounded to
    # 128 (gather-group granularity; an odd trailing group uses plain
    # matmuls instead of a DoubleRow pair)
    cap = ((cnt.max(axis=0) + 127) // 128) * 128               # [4, 8]
    offs_flat = np.concatenate([[0], np.cumsum(cap.ravel())])
    E_pad = int(offs_flat[-1])
    Gg = E_pad // 128                                          # 128-groups

    # group -> psum bank metadata (uniform across cores). DoubleRow matmuls
    # require PSUM base partition 0, so superblock sb owns rows 0:64, cols
    # 0:256 of bank sb.
    group_meta = []
    for b in range(N_BUCKETS):
        for sb in range(NSB):
            group_meta.extend([sb] * (int(cap[b, sb]) // 128))
    assert len(group_meta) == Gg

    # chunk list in SUPERBLOCK-MAJOR order across buckets so each
    # superblock's accumulation closes (and its epilogue runs) while later
    # superblocks are still gathering.  chunks[sb] = [(start, n, bucket)];
    # every chunk has an even group count except the last chunk of an
    # odd-group run (its final group is emitted as plain matmuls).
    bucket_bounds = []
    for b in range(N_BUCKETS):
        bucket_bounds.append((int(offs_flat[b * NSB]), int(cap[b].sum())))
    chunks = []
    for sb in range(NSB):
        sb_chunks = []
        for b in range(N_BUCKETS):
            s = int(offs_flat[b * NSB + sb])
            nrun = int(cap[b, sb])
            ng = nrun // 128
            ncalls = max(1, (nrun + CHUNK - 1) // CHUNK)
            base_p, extra = divmod(ng // 2, ncalls)
            pieces = [2 * (base_p + (1 if c < extra else 0))
                      for c in range(ncalls)]
            pieces[-1] += ng % 2
            for g in pieces:
                if g == 0:
                    continue
                n = g * 128
                sb_chunks.append((s, n, b))
                s += n
        chunks.append(sb_chunks)

    meta = dict(E_pad=E_pad, G2=Gg, chunks=chunks, group_meta=group_meta,
                bucket_bounds=bucket_bounds)

    # --- per-core device arrays ---
    def wrap16(x):   # entry e -> [e%16, e//16], replicated to 128 partitions
        w = x.reshape(-1, 16).T
        return np.ascontiguousarray(np.tile(w, (8, 1)))

    in_maps = []
    for m in range(N_CORES):
        it, sl, av, key, rep, first_of = percore[m]
        ne = it.shape[0]
        # slots go to representatives (in run-stable order); every entry
        # maps to its representative's slot
        gstart = np.concatenate([[0], np.cumsum(cnt[m].ravel())])
        nrep = int(rep.sum())
        krep = key[rep]
        rank = np.arange(nrep) - gstart[krep]
        slot_rep = offs_flat[krep] + rank
        srep = np.zeros(ne, np.int64)
        srep[rep] = slot_rep
        slot = srep[first_of]

        lidx = np.zeros(E_pad, np.int16)
        lidx[slot_rep] = (it[rep] - (it[rep] // BUCKET_ROWS)
                          * BUCKET_ROWS).astype(np.int16)

        # lhsT coefficients, group-major: [128, Gg, 2, 64] = (Sp, S) per
        # 128-group; accumulated so merged duplicates get multi-hot columns
        g = slot // 128
        part = slot % 128
        col = sl - SB * (key % NSB)
        LTf = np.zeros((128, Gg, 2, SB), np.float32)
        np.add.at(LTf, (part, g, 0, col), av)
        np.add.at(LTf, (part, g, 1, col), 1.0)

        in_maps.append({
            "xy": XYs,
            "idx16": wrap16(lidx),
            "lt": LTf.astype(FP8),
            "qn2": np.ascontiguousarray(Qn2[m * SEGS_PER_CORE:(m + 1) * SEGS_PER_CORE]),
            "bui": np.ascontiguousarray(bui[m * SEGS_PER_CORE:(m + 1) * SEGS_PER_CORE]),
        })
    return in_maps, meta


def _build_graph(meta):
    from concourse import bacc, mybir
    from concourse.tile import TileContext

    E_pad, Gg = meta["E_pad"], meta["G2"]
    chunks, group_meta = meta["chunks"], meta["group_meta"]
    bucket_bounds = meta["bucket_bounds"]

    nc = bacc.Bacc("TRN2", target_bir_lowering=False, debug=False,
                   num_devices=N_CORES, num_swdge_queues=N_QUEUES)
    fp8, f32, i16 = mybir.dt.float8e4, mybir.dt.float32, mybir.dt.int16
    DR = mybir.MatmulPerfMode.DoubleRow

    xy_d = nc.declare_dram_parameter("xy", [NUM_ITEMS, 256], fp8, isOutput=False)
    idx_d = nc.declare_dram_parameter("idx16", [128, E_pad // 16], i16, isOutput=False)
    lt_d = nc.declare_dram_parameter("lt", [128, Gg, 2, SB], fp8, isOutput=False)
    qn_d = nc.declare_dram_parameter("qn2", [SEGS_PER_CORE, 256], f32, isOutput=False)
    bui_d = nc.declare_dram_parameter("bui", [SEGS_PER_CORE], f32, isOutput=False)
    out_d = nc.declare_dram_parameter("out", [SEGS_PER_CORE], f32, isOutput=True)

    # 8 PSUM bank tiles; superblock sb lives at rows 0:64, cols 0:256 of
    # bank sb.
    n_tiles = NSB  # 8

    with TileContext(nc) as tc:
        with (
            tc.tile_pool(name="const", bufs=1) as cpool,
            tc.tile_pool(name="xy", bufs=24) as xypool,
            tc.tile_pool(name="lt", bufs=8) as lpool,
            tc.tile_pool(name="epi", bufs=2) as epool,
            tc.tile_pool(name="psum", bufs=1, space="PSUM") as ppool,
        ):
            # per-bucket idx tiles on the sync HWDGE queue (they gate the
            # gathers).  Bucket 0's idx is split so the first gather calls
            # aren't gated on a large load.  lhsT is streamed per-chunk (it
            # no longer fits SBUF at SB=128).
            idx_tiles = []
            HEAD = 4096
            # all four HEAD loads first (they gate the first gather wave),
            # then the big rests
            for b in range(N_BUCKETS):
                boff, bn = bucket_bounds[b]
                if bn == 0:
                    idx_tiles.append(None)
                    continue
                t = cpool.tile([128, bn // 16], i16, tag=f"idx{b}")
                nc.sync.dma_start(
                    out=t[:, 0:min(HEAD, bn) // 16],
                    in_=idx_d[:, boff // 16:(boff + min(HEAD, bn)) // 16])
                idx_tiles.append(t)
            for b in range(N_BUCKETS):
                boff, bn = bucket_bounds[b]
                if bn > HEAD:
                    nc.sync.dma_start(
                        out=idx_tiles[b][:, HEAD // 16:bn // 16],
                        in_=idx_d[:, (boff + HEAD) // 16:(boff + bn) // 16])

            # epilogue inputs preloaded upfront (no dependencies)
            qn_t = cpool.tile([128, NSB, 256], f32, tag="qn")
            bui_t = cpool.tile([128, NSB], f32, tag="bui")
            for sb in range(NSB):
                nc.scalar.dma_start(
                    out=qn_t[0:SB, sb, :], in_=qn_d[SB * sb:SB * sb + SB, :])
                nc.scalar.dma_start(
                    out=bui_t[0:SB, sb:sb + 1],
                    in_=bui_d[SB * sb:SB * sb + SB])

            zeros_t = cpool.tile([128, 512], fp8, tag="zeros")
            nc.vector.memset(zeros_t[:], 0.0)

            psum_t = []
            for kbank in range(n_tiles):
                pt = ppool.tile([128, 512], f32, tag=f"bank{kbank}")
                psum_t.append(pt)
                nc.tensor.matmul(
                    out=pt[0:SB, 0:512], lhsT=zeros_t[:, 0:SB],
                    rhs=zeros_t[:, 0:512], start=True, stop=False,
                )

            ci = 0
            for sb in range(NSB):
                for (start, n, b) in chunks[sb]:
                    nG = n // 128
                    boff = bucket_bounds[b][0]
                    xyt = xypool.tile([128, nG, 256], fp8, tag="xyt")
                    nc.gpsimd.dma_gather(
                        out_ap=xyt[:],
                        in_ap=xy_d[b * BUCKET_ROWS:(b + 1) * BUCKET_ROWS, :],
                        idxs_ap=idx_tiles[b][:, (start - boff) // 16:
                                             (start - boff + n) // 16],
                        num_idxs=n,
                        num_idxs_reg=n,
                        elem_size=256,
                        single_packet=False,
                        queue_num=ci % N_QUEUES,
                    )
                    ci += 1
                    ltc = lpool.tile([128, nG, 2, SB], fp8, tag="ltc")
                    nc.scalar.dma_start(
                        out=ltc[:],
                        in_=lt_d[:, start // 128:(start + n) // 128, :, :])
                    u = 0
                    while u < nG:
                        Gi = start // 128 + u
                        bank = group_meta[Gi]
                        if u + 1 < nG:
                            # DoubleRow pair: k-tiles = groups u, u+1
                            for which in (0, 1):
                                c0, c1 = 128 * which, 128 * (which + 1)
                                nc.tensor.matmul(
                                    out=psum_t[bank][0:SB, c0:c1],
                                    lhsT=ltc[:, u:u + 2, which, :],
                                    rhs=xyt[:, u:u + 2, c0:c1],
                                    start=False, stop=False,
                                    perf_mode=DR,
                                )
                            u += 2
                        else:
                            # odd trailing group: plain fp8 matmuls
                            for which in (0, 1):
                                c0, c1 = 128 * which, 128 * (which + 1)
                                nc.tensor.matmul(
                                    out=psum_t[bank][0:SB, c0:c1],
                                    lhsT=ltc[:, u, which, :],
                                    rhs=xyt[:, u, c0:c1],
                                    start=False, stop=False,
                                )
                            u += 1

                # this superblock's bank is done: close its accumulation
                # group (zero-region = full 2KB bank) and run its epilogue
                # while later superblocks are still gathering.
                nc.tensor.matmul(
                    out=psum_t[sb][0:SB, 0:512], lhsT=zeros_t[:, 0:SB],
                    rhs=zeros_t[:, 0:512], start=False, stop=True,
                )
                s0 = SB * sb
                prod_t = epool.tile([128, 256], f32, tag="prod")
                nc.vector.tensor_tensor(
                    out=prod_t[0:SB, :],
                    in0=psum_t[sb][0:SB, 0:256],
                    in1=qn_t[0:SB, sb, :],
                    op=mybir.AluOpType.mult,
                )
                red_t = epool.tile([128, 1], f32, tag="red")
                nc.vector.tensor_reduce(
                    out=red_t[0:SB, 0:1], in_=prod_t[0:SB, :],
                    axis=mybir.AxisListType.X,
                    op=mybir.AluOpType.add,
                )
                nc.vector.tensor_add(red_t[0:SB, 0:1], red_t[0:SB, 0:1],
                                     bui_t[0:SB, sb:sb + 1])
                nc.sync.dma_start(
                    out=out_d[s0:s0 + SB], in_=red_t[0:SB, 0:1])

    nc.compile()
    return nc


def kernel(bu, bi, Q, X, Y, user, item, imp_items, imp_ratings, segment_ids,
           _sim=False):
    bu = np.asarray(bu, np.float32)
    bi = np.asarray(bi, np.float32)
    Q = np.asarray(Q, np.float32)
    X = np.asarray(X, np.float32)
    Y = np.asarray(Y, np.float32)
    user = np.asarray(user).astype(np.int64)
    item = np.asarray(item).astype(np.int64)
    imp_items = np.asarray(imp_items).astype(np.int64)
    imp_ratings = np.asarray(imp_ratings).astype(np.int64)
    segment_ids = np.asarray(segment_ids).astype(np.int64)

    in_maps, meta = _host_prep(bu, bi, Q, X, Y, user, item, imp_items,
                               imp_ratings, segment_ids)
    nc = _build_graph(meta)

    if _sim:
        from concourse import bass_interp
        sim = bass_interp.CoreSim(nc)
        sim.assign_tensors(in_maps[0])
        sim.simulate()
        out0 = np.array(sim.tensor("out"))
        return sim, out0, in_maps, meta

    from concourse.bass_utils import run_bass_kernel_spmd
    res = run_bass_kernel_spmd(nc, in_maps, core_ids=list(range(N_CORES)),
                               trace=False)
    out = np.concatenate([res.results[m]["out"] for m in range(N_CORES)])
    return out.astype(np.float32)



# revision 5
# speedup vs baseline: 1.0539x; 1.0539x over previous
"""AsymmetricSVD segment-reduce kernel for 8 TRN2 NeuronCores.

Strategy (data-parallel over segments, fp8 + DoubleRow + pair-descriptors):
  - Core m owns segments [512m, 512(m+1)) and their contiguous implicit
    entries (segment_ids is sorted).
  - Host precomputes per-entry scalar a_e = r_e - MU - bu[user[seg_e]] and a
    fused fp8 table XY = 128*[X | Y - bi*X] (so w*X + Y == a*X + Y'; the
    2^7 scale keeps fp8e4 out of subnormals and is folded back in Qn2).
  - Entries are bucketed by item range (4 buckets of 25000 rows so gather
    indices fit int16) and grouped by 128-segment superblock (4 per core).
    Dedup at (bucket, superblock) scope: duplicate items merge into
    multi-hot lhsT columns (one gather descriptor per unique item).
  - PAIR TRICK: the SWDGE wall is ~2.2ns per descriptor regardless of
    size, so unique items whose aligned partner (item^1) is also present
    in the run are fetched by ONE 512B descriptor from a second [50000,
    512] view of the same table.  Remaining singles use 256B descriptors.
    Each run therefore has a pair-slot region and a single-slot region,
    both padded to 128 (cross-core max, so the compiled graph is uniform).
  - Device gathers via gpsimd.dma_gather (SWDGE), 4 queues, calls sized
    ~<=1664 descriptors (under the ~2048-desc ring capacity).
  - lhsT coefficient tiles are PRE-BUILT ON HOST in fp8 and streamed
    per-chunk (group-major).  Single stream: [128, G, 2, 128] = (Sp, S)
    planes.  Pair stream: [128, G, 4, 128] = (Sp0, S0, Sp1, S1) planes
    for the even/odd subrow of each 512B slot.
  - Adjacent group pairs run as fp8 DoubleRow matmuls (2 k-tiles); odd
    trailing groups use plain fp8 matmuls:
      single: PSUM[sb][0:128,   0:128] += sum a_e*X   (plane Sp)
              PSUM[sb][0:128, 128:256] += sum (Y')    (plane S)
      pair:   same, x2 for subrows 0/1 (rhs cols sub*256+...).
  - Chunks run superblock-major across buckets, so each superblock's
    accumulation closes and its epilogue runs while later superblocks are
    still gathering.
  - Epilogue: rui[seg] = bui[seg] + reduce_add(PSUM[seg, 0:256] * Qn2[seg])
    with Qn2 = [Qn | Qn], Qn = Q[item]*norm/128 precomputed on host.
"""

import numpy as np
import ml_dtypes

MU = 3.5
B = 4096
F = 128
NUM_ITEMS = 100000
N_CORES = 8
SEGS_PER_CORE = B // N_CORES            # 512
N_BUCKETS = 4
BUCKET_ROWS = (NUM_ITEMS + N_BUCKETS - 1) // N_BUCKETS   # 25000 < 32768 (int16)
SB = 128                                 # segments per superblock
NSB = SEGS_PER_CORE // SB                # 4 superblocks per core
CHUNK = 1664                             # max descriptors per gather call
N_QUEUES = 4                             # SWDGE gather queues (ucode max)
FP8 = ml_dtypes.float8_e4m3
XSCALE = 128.0                           # fp8 range scale for X/Y' rows
NRUN = N_BUCKETS * NSB                   # runs per core


def _round128(x):
    return (x + 127) // 128 * 128


def _host_prep(bu, bi, Q, X, Y, user, item, imp_items, imp_ratings, segment_ids):
    """All index/scalar preprocessing. Returns per-core device arrays and
    uniform cross-core metadata for codegen."""
    a_full = imp_ratings.astype(np.float32) - MU - bu[user[segment_ids], 0]
    Yp = Y - bi * X                                    # [NUM_ITEMS, F]
    XYs = np.concatenate([X * XSCALE, Yp * XSCALE], axis=1).astype(FP8)

    counts = np.bincount(segment_ids, minlength=B).astype(np.float32)
    norm = np.where(counts > 0, counts, 1.0) ** -0.5
    bui = (MU + bu[user, 0] + bi[item, 0]).astype(np.float32)          # [B]
    Qh = (Q[item] * (norm / XSCALE)[:, None]).astype(np.float32)       # [B, F]
    Qn2 = np.concatenate([Qh, Qh], axis=1)                             # [B, 256]

    bounds = np.searchsorted(segment_ids, np.arange(0, B + 1, SEGS_PER_CORE))

    # --- per-core, per-run slot construction.  percore[m][r] holds
    # (pair_pks, single_items, entry maps) in first-appearance order.
    percore = []
    cnt_p = np.zeros((N_CORES, NRUN), np.int64)
    cnt_s = np.zeros((N_CORES, NRUN), np.int64)
    for m in range(N_CORES):
        lo, hi = bounds[m], bounds[m + 1]
        it = imp_items[lo:hi]
        sl = (segment_ids[lo:hi] - m * SEGS_PER_CORE).astype(np.int64)
        av = a_full[lo:hi]
        bk = it // BUCKET_ROWS
        key = bk * NSB + sl // SB
        order = np.argsort(key, kind="stable")
        it, sl, av, key = it[order], sl[order], av[order], key[order]
        rb = np.searchsorted(key, np.arange(NRUN + 1))
        runs = []
        for r in range(NRUN):
            rit = it[rb[r]:rb[r + 1]]
            rsl = sl[rb[r]:rb[r + 1]]
            rav = av[rb[r]:rb[r + 1]]
            uniq, first_pos, inv = np.unique(rit, return_index=True,
                                             return_inverse=True)
            paired = np.isin(uniq ^ 1, uniq)
            # pair slots: unique pk = item//2, appearance order by the
            # earlier of the two members' first occurrence
            pk = uniq[paired] // 2
            upk, pk_inv = np.unique(pk, return_inverse=True)
            pk_first = np.full(upk.shape, 1 << 30, np.int64)
            np.minimum.at(pk_first, pk_inv, first_pos[paired])
            p_order = np.argsort(pk_first, kind="stable")
            p_rank = np.empty_like(p_order)
            p_rank[p_order] = np.arange(upk.shape[0])
            # single slots in appearance order
            s_items = uniq[~paired]
            s_first = first_pos[~paired]
            s_order = np.argsort(s_first, kind="stable")
            s_rank = np.empty_like(s_order)
            s_rank[s_order] = np.arange(s_items.shape[0])
            # per-unique routing: slot rank within run + stream + subrow
            u_stream = paired                  # True -> pair stream
            u_rank = np.empty(uniq.shape[0], np.int64)
            u_rank[paired] = p_rank[pk_inv]
            u_rank[~paired] = s_rank
            u_sub = uniq & 1
            runs.append(dict(
                n=rit.shape[0], inv=inv, rsl=rsl, rav=rav,
                upk=upk[p_order], s_items=s_items[s_order],
                u_stream=u_stream, u_rank=u_rank, u_sub=u_sub))
            cnt_p[m, r] = upk.shape[0]
            cnt_s[m, r] = s_items.shape[0]
        percore.append(runs)

    cap_p = _round128(cnt_p.max(axis=0))               # [NRUN]
    cap_s = _round128(cnt_s.max(axis=0))
    offs_p = np.concatenate([[0], np.cumsum(cap_p)])
    offs_s = np.concatenate([[0], np.cumsum(cap_s)])
    Ep, Es = int(offs_p[-1]), int(offs_s[-1])
    Gp, Gs = Ep // 128, Es // 128

    # group -> psum bank (uniform across cores); run r = b*NSB + sb
    gm_p, gm_s = [], []
    for r in range(NRUN):
        gm_p.extend([r % NSB] * (int(cap_p[r]) // 128))
        gm_s.extend([r % NSB] * (int(cap_s[r]) // 128))

    # chunk list in SUPERBLOCK-MAJOR order across buckets; per run the
    # pair chunks come first.  chunks[sb] = [(stream, start, n, b)].
    def _pieces(nrun):
        ng = nrun // 128
        ncalls = max(1, (nrun + CHUNK - 1) // CHUNK)
        base, extra = divmod(ng // 2, ncalls)
        ps = [2 * (base + (1 if c < extra else 0)) for c in range(ncalls)]
        ps[-1] += ng % 2
        return [p for p in ps if p > 0]

    chunks = []
    for sb in range(NSB):
        sb_chunks = []
        for b in range(N_BUCKETS):
            r = b * NSB + sb
            for stream, offs, cap in (("p", offs_p, cap_p),
                                      ("s", offs_s, cap_s)):
                s0 = int(offs[r])
                for g in _pieces(int(cap[r])):
                    sb_chunks.append((stream, s0, g * 128, b))
                    s0 += g * 128
        chunks.append(sb_chunks)

    # per-bucket region bounds for idx tiles (bucket-major contiguity)
    bb_p = [(int(offs_p[b * NSB]), int(cap_p[b * NSB:(b + 1) * NSB].sum()))
            for b in range(N_BUCKETS)]
    bb_s = [(int(offs_s[b * NSB]), int(cap_s[b * NSB:(b + 1) * NSB].sum()))
            for b in range(N_BUCKETS)]

    meta = dict(Ep=Ep, Es=Es, Gp=Gp, Gs=Gs, chunks=chunks,
                gm_p=gm_p, gm_s=gm_s, bb_p=bb_p, bb_s=bb_s)

    # --- per-core device arrays ---
    def wrap16(x):   # entry e -> [e%16, e//16], replicated to 128 partitions
        w = x.reshape(-1, 16).T
        return np.ascontiguousarray(np.tile(w, (8, 1)))

    in_maps = []
    for m in range(N_CORES):
        idx_p = np.zeros(Ep, np.int16)
        idx_s = np.zeros(Es, np.int16)
        LT2f = np.zeros((128, Gp, 4, SB), np.float32)
        LTf = np.zeros((128, Gs, 2, SB), np.float32)
        for r in range(NRUN):
            b = r // NSB
            d = percore[m][r]
            # idx values (local to bucket)
            npk = d["upk"].shape[0]
            idx_p[offs_p[r]:offs_p[r] + npk] = (
                d["upk"] - b * (BUCKET_ROWS // 2)).astype(np.int16)
            nsi = d["s_items"].shape[0]
            idx_s[offs_s[r]:offs_s[r] + nsi] = (
                d["s_items"] - b * BUCKET_ROWS).astype(np.int16)
            # entry -> (global slot, subrow) ; accumulate lhsT planes
            inv, rsl, rav = d["inv"], d["rsl"], d["rav"]
            col = rsl - SB * (r % NSB)
            e_stream = d["u_stream"][inv]
            e_rank = d["u_rank"][inv]
            e_sub = d["u_sub"][inv]
            # pair entries
            pe = e_stream
            gslot = offs_p[r] + e_rank[pe]
            g, part = gslot // 128, gslot % 128
            np.add.at(LT2f, (part, g, 2 * e_sub[pe] + 0, col[pe]), rav[pe])
            np.add.at(LT2f, (part, g, 2 * e_sub[pe] + 1, col[pe]), 1.0)
            # single entries
            se = ~e_stream
            gslot = offs_s[r] + e_rank[se]
            g, part = gslot // 128, gslot % 128
            np.add.at(LTf, (part, g, 0, col[se]), rav[se])
            np.add.at(LTf, (part, g, 1, col[se]), 1.0)

        in_maps.append({
            "xy": XYs,
            "xy2": XYs.reshape(NUM_ITEMS // 2, 512),
            "idxp": wrap16(idx_p),
            "idxs": wrap16(idx_s),
            "ltp": LT2f.astype(FP8),
            "lts": LTf.astype(FP8),
            "qn2": np.ascontiguousarray(Qn2[m * SEGS_PER_CORE:(m + 1) * SEGS_PER_CORE]),
            "bui": np.ascontiguousarray(bui[m * SEGS_PER_CORE:(m + 1) * SEGS_PER_CORE]),
        })
    return in_maps, meta


def _build_graph(meta):
    from concourse import bacc, mybir
    from concourse.tile import TileContext

    Ep, Es, Gp, Gs = meta["Ep"], meta["Es"], meta["Gp"], meta["Gs"]
    chunks, gm_p, gm_s = meta["chunks"], meta["gm_p"], meta["gm_s"]
    bb_p, bb_s = meta["bb_p"], meta["bb_s"]

    nc = bacc.Bacc("TRN2", target_bir_lowering=False, debug=False,
                   num_devices=N_CORES, num_swdge_queues=N_QUEUES)
    fp8, f32, i16 = mybir.dt.float8e4, mybir.dt.float32, mybir.dt.int16
    DR = mybir.MatmulPerfMode.DoubleRow

    xy_d = nc.declare_dram_parameter("xy", [NUM_ITEMS, 256], fp8, isOutput=False)
    xy2_d = nc.declare_dram_parameter("xy2", [NUM_ITEMS // 2, 512], fp8,
                                      isOutput=False)
    idxp_d = nc.declare_dram_parameter("idxp", [128, Ep // 16], i16, isOutput=False)
    idxs_d = nc.declare_dram_parameter("idxs", [128, Es // 16], i16, isOutput=False)
    ltp_d = nc.declare_dram_parameter("ltp", [128, Gp, 4, SB], fp8, isOutput=False)
    lts_d = nc.declare_dram_parameter("lts", [128, Gs, 2, SB], fp8, isOutput=False)
    qn_d = nc.declare_dram_parameter("qn2", [SEGS_PER_CORE, 256], f32, isOutput=False)
    bui_d = nc.declare_dram_parameter("bui", [SEGS_PER_CORE], f32, isOutput=False)
    out_d = nc.declare_dram_parameter("out", [SEGS_PER_CORE], f32, isOutput=True)

    with TileContext(nc) as tc:
        with (
            tc.tile_pool(name="const", bufs=1) as cpool,
            tc.tile_pool(name="xy", bufs=16) as xypool,
            tc.tile_pool(name="xy2", bufs=8) as xy2pool,
            tc.tile_pool(name="lt", bufs=6) as lpool,
            tc.tile_pool(name="lt2", bufs=6) as l2pool,
            tc.tile_pool(name="epi", bufs=2) as epool,
            tc.tile_pool(name="psum", bufs=1, space="PSUM") as ppool,
        ):
            # per-bucket idx tiles on the sync HWDGE queue (they gate the
            # gathers).  The first chunk's worth of pair-idx for bucket 0
            # loads first so the first gather isn't gated on a large load.
            idxp_tiles, idxs_tiles = [], []
            HEAD = 2048
            for b in range(N_BUCKETS):
                boff, bn = bb_p[b]
                t = cpool.tile([128, max(bn, 16) // 16], i16, tag=f"idxp{b}")
                if bn:
                    nc.sync.dma_start(
                        out=t[:, 0:min(HEAD, bn) // 16],
                        in_=idxp_d[:, boff // 16:(boff + min(HEAD, bn)) // 16])
                idxp_tiles.append(t)
            for b in range(N_BUCKETS):
                boff, bn = bb_s[b]
                t = cpool.tile([128, max(bn, 16) // 16], i16, tag=f"idxs{b}")
                if bn:
                    nc.sync.dma_start(
                        out=t[:, 0:min(HEAD, bn) // 16],
                        in_=idxs_d[:, boff // 16:(boff + min(HEAD, bn)) // 16])
                idxs_tiles.append(t)
            for b in range(N_BUCKETS):
                boff, bn = bb_p[b]
                if bn > HEAD:
                    nc.sync.dma_start(
                        out=idxp_tiles[b][:, HEAD // 16:bn // 16],
                        in_=idxp_d[:, (boff + HEAD) // 16:(boff + bn) // 16])
                boff, bn = bb_s[b]
                if bn > HEAD:
                    nc.sync.dma_start(
                        out=idxs_tiles[b][:, HEAD // 16:bn // 16],
                        in_=idxs_d[:, (boff + HEAD) // 16:(boff + bn) // 16])

            # epilogue inputs preloaded upfront (no dependencies)
            qn_t = cpool.tile([128, NSB, 256], f32, tag="qn")
            bui_t = cpool.tile([128, NSB], f32, tag="bui")
            for sb in range(NSB):
                nc.scalar.dma_start(
                    out=qn_t[0:SB, sb, :], in_=qn_d[SB * sb:SB * sb + SB, :])
                nc.scalar.dma_start(
                    out=bui_t[0:SB, sb:sb + 1],
                    in_=bui_d[SB * sb:SB * sb + SB])

            zeros_t = cpool.tile([128, 512], fp8, tag="zeros")
            nc.vector.memset(zeros_t[:], 0.0)

            psum_t = []
            for kbank in range(NSB):
                pt = ppool.tile([128, 512], f32, tag=f"bank{kbank}")
                psum_t.append(pt)
                nc.tensor.matmul(
                    out=pt[0:SB, 0:512], lhsT=zeros_t[:, 0:SB],
                    rhs=zeros_t[:, 0:512], start=True, stop=False,
                )

            ci = 0
            for sb in range(NSB):
                for (stream, start, n, b) in chunks[sb]:
                    nG = n // 128
                    if stream == "p":
                        boff = bb_p[b][0]
                        xyt = xy2pool.tile([128, nG, 512], fp8, tag="xyt2")
                        nc.gpsimd.dma_gather(
                            out_ap=xyt[:],
                            in_ap=xy2_d[b * (BUCKET_ROWS // 2):
                                        (b + 1) * (BUCKET_ROWS // 2), :],
                            idxs_ap=idxp_tiles[b][:, (start - boff) // 16:
                                                  (start - boff + n) // 16],
                            num_idxs=n, num_idxs_reg=n,
                            elem_size=512, single_packet=False,
                            queue_num=ci % N_QUEUES,
                        )
                        ci += 1
                        ltc = l2pool.tile([128, nG, 4, SB], fp8, tag="ltc2")
                        nc.scalar.dma_start(
                            out=ltc[:],
                            in_=ltp_d[:, start // 128:(start + n) // 128, :, :])
                        planes = 4
                        gm = gm_p
                    else:
                        boff = bb_s[b][0]
                        xyt = xypool.tile([128, nG, 256], fp8, tag="xyt")
                        nc.gpsimd.dma_gather(
                            out_ap=xyt[:],
                            in_ap=xy_d[b * BUCKET_ROWS:(b + 1) * BUCKET_ROWS, :],
                            idxs_ap=idxs_tiles[b][:, (start - boff) // 16:
                                                  (start - boff + n) // 16],
                            num_idxs=n, num_idxs_reg=n,
                            elem_size=256, single_packet=False,
                            queue_num=ci % N_QUEUES,
                        )
                        ci += 1
                        ltc = lpool.tile([128, nG, 2, SB], fp8, tag="ltc")
                        nc.scalar.dma_start(
                            out=ltc[:],
                            in_=lts_d[:, start // 128:(start + n) // 128, :, :])
                        planes = 2
                        gm = gm_s
                    bank = gm[start // 128]
                    u = 0
                    while u < nG:
                        pair = u + 1 < nG
                        for sub in range(planes // 2):
                            for which in (0, 1):
                                c0 = 256 * sub + 128 * which
                                o0 = 128 * which
                                if pair:
                                    nc.tensor.matmul(
                                        out=psum_t[bank][0:SB, o0:o0 + 128],
                                        lhsT=ltc[:, u:u + 2, 2 * sub + which, :],
                                        rhs=xyt[:, u:u + 2, c0:c0 + 128],
                                        start=False, stop=False,
                                        perf_mode=DR,
                                    )
                                else:
                                    nc.tensor.matmul(
                                        out=psum_t[bank][0:SB, o0:o0 + 128],
                                        lhsT=ltc[:, u, 2 * sub + which, :],
                                        rhs=xyt[:, u, c0:c0 + 128],
                                        start=False, stop=False,
                                    )
                        u += 2 if pair else 1

                # this superblock's bank is done: close its accumulation
                # group (zero-region = full 2KB bank) and run its epilogue
                # while later superblocks are still gathering.
                nc.tensor.matmul(
                    out=psum_t[sb][0:SB, 0:512], lhsT=zeros_t[:, 0:SB],
                    rhs=zeros_t[:, 0:512], start=False, stop=True,
                )
                s0 = SB * sb
                prod_t = epool.tile([128, 256], f32, tag="prod")
                nc.vector.tensor_tensor(
                    out=prod_t[0:SB, :],
                    in0=psum_t[sb][0:SB, 0:256],
                    in1=qn_t[0:SB, sb, :],
                    op=mybir.AluOpType.mult,
                )
                red_t = epool.tile([128, 1], f32, tag="red")
                nc.vector.tensor_reduce(
                    out=red_t[0:SB, 0:1], in_=prod_t[0:SB, :],
                    axis=mybir.AxisListType.X,
                    op=mybir.AluOpType.add,
                )
                nc.vector.tensor_add(red_t[0:SB, 0:1], red_t[0:SB, 0:1],
                                     bui_t[0:SB, sb:sb + 1])
                nc.sync.dma_start(
                    out=out_d[s0:s0 + SB], in_=red_t[0:SB, 0:1])

    nc.compile()
    return nc


def kernel(bu, bi, Q, X, Y, user, item, imp_items, imp_ratings, segment_ids,
           _sim=False):
    bu = np.asarray(bu, np.float32)
    bi = np.asarray(bi, np.float32)
    Q = np.asarray(Q, np.float32)
    X = np.asarray(X, np.float32)
    Y = np.asarray(Y, np.float32)
    user = np.asarray(user).astype(np.int64)
    item = np.asarray(item).astype(np.int64)
    imp_items = np.asarray(imp_items).astype(np.int64)
    imp_ratings = np.asarray(imp_ratings).astype(np.int64)
    segment_ids = np.asarray(segment_ids).astype(np.int64)

    in_maps, meta = _host_prep(bu, bi, Q, X, Y, user, item, imp_items,
                               imp_ratings, segment_ids)
    nc = _build_graph(meta)

    if _sim:
        from concourse import bass_interp
        sim = bass_interp.CoreSim(nc)
        sim.assign_tensors(in_maps[0])
        sim.simulate()
        out0 = np.array(sim.tensor("out"))
        return sim, out0, in_maps, meta

    from concourse.bass_utils import run_bass_kernel_spmd
    res = run_bass_kernel_spmd(nc, in_maps, core_ids=list(range(N_CORES)),
                               trace=False)
    out = np.concatenate([res.results[m]["out"] for m in range(N_CORES)])
    return out.astype(np.float32)
